# revision 19
# baseline (speedup 1.0000x reference)
"""DeformableAttention1D on 8 TRN2 NeuronCores via Bass/Tile.

Sharding: core c handles offset-group g=c//2 (64 of 256 channels, 2 of 8 heads)
and query-half qh=c%2 (512 of 1024 positions). Each core computes its group's
offsets/gather/bias/attention independently; the final output projection is
computed as a partial (wo sliced by group) and summed on the host (the
"all-reduce" of the output projection).

Key idea vs the straightforward implementation: the CPB relative-position-bias
MLP is a scalar->2 function F(d) of the signed distance d = gq_i - vgsp1_j,
and the query grid steps uniformly by h_q = 2/1023. Tabulating F on a grid
with spacing exactly h_q (a host-side weights-only precompute, like any other
weight repacking) turns the bias into

    bias[i, j] = (1-r_j) T[m_j + i] + r_j T[m_j + 1 + i],   m_j + r_j = c_j,

i.e. a per-column shifted window of the table. On device this is 9 matmuls of
"tent" interpolation one-hots max(0, 1-|c_j - kappa|) (stationary) against
host-precomputed Hankel slabs of T (moving), accumulated directly into the
attention-logit PSUM on top of q.k — the bias costs zero vector-engine work
in the attention phase. The tent matrices also implement the bilinear kv
grid-sample gather (zeros padding included). Tents are only materialized in
their static nonzero j-windows (~43 of 256 columns per kappa-tile).

Device numerics: fp32 data, fp32r matmuls (1 cycle/col vs 4 for fp32). The
ACT engine is restricted to ONE table set (natural_log_exp_and_others:
Exp/Ln/Relu/Copy/Identity/Square); tanh and erf(gelu) are composed from
Exp + DVE ops. Max bias interp error ~3e-4 in the logits.
"""
import os
import sys

sys.path.insert(0, "/opt/trn_rl_repo")

import numpy as np

import concourse.bacc as bacc
import concourse.bass as bass
import concourse.mybir as mybir
import concourse.tile as tile
import concourse.bass_utils as bass_utils

F32 = mybir.dt.float32
F32R = mybir.dt.float32r
I32 = mybir.dt.int32
U32 = mybir.dt.uint32
AF = mybir.ActivationFunctionType
ALU = mybir.AluOpType

# model dims (hardcoded per problem spec)
DIM = 256
N = 1024
G = 4
HEADS = 8
DH = 32
NDS = 256          # downsampled kv positions
QS = 512           # queries per core
DPG = 64           # channels per group
OFF_K = 6
DS = 4             # downsample stride
OFF_SCALE = 4.0
NCORES = 8

# bias lookup table
HQ = 2.0 / 1023.0  # query grid step == table spacing
CQ = 1040.0        # index offset so c_j = CQ - vgsp1_j/HQ stays in [0.9, 1057]
LTAB = 1664        # table length (slabs need up to 128*8+127+511 = 1662)
NTT = 9            # tent kappa-tiles (c_j+1 < 1152)
NT = 8             # x position tiles for the kv gather
A_S = 1024.0 / 255.0   # d ppix / d vg_raw  (|offset| < 4 -> +-4*A_S slack)
A_O = 1023.0 / 255.0   # -d c / d vg_raw

# A&S 7.1.25 3-term erf coefficients (|err| <= 2.5e-5)
ERF_P3 = 0.47047
ERF_A3 = [0.3480242, -0.0958798, 0.7478556]

# PE warmup chain lengths (tuned against the cost-model timeline)
WARM_A = 6
WARM_B = 12
WARM_C = 7

_CACHED = {}


def _s_window(t):
    """Static j-range where the kv-gather tent for position tile t can be
    nonzero: ppix_j in (128t-1, 128t+128), ppix = A_S*vg_raw-0.5, vg_raw in
    (j-4, j+4)."""
    jlo = max(0, int(np.floor((128 * t - 0.5) / A_S - 4)) - 1)
    jhi = min(NDS, int(np.ceil((128 * t + 128.5) / A_S + 4)) + 1)
    return jlo, jhi


def _o_window(t):
    """Static j-range where the bias tent for kappa tile t can be nonzero:
    c_j in (128t-1, 128t+128), c = CQ - A_O*vg_raw."""
    jlo = max(0, int(np.floor((912 - 128 * t) / A_O - 4)) - 1)
    jhi = min(NDS, int(np.ceil((1041 - 128 * t) / A_O + 4)) + 1)
    return jlo, jhi


def _patch_act_tables():
    """Restrict activation-table selection to the single set that covers all
    ACT functions used by this kernel, so exactly one table load is emitted
    (runtime table swaps do not work in this environment)."""
    import concourse.hw_specs as hw_specs

    if getattr(bacc, "_deform_act_patch", False):
        return
    orig = hw_specs.get_activation_tables

    keep = "natural_log_exp_and_others"

    def patched(module_arch):
        tabs = orig(module_arch)
        keep_funcs = tabs[keep]
        out = {}
        for name, funcs in tabs.items():
            if name == keep:
                out[name] = funcs
            else:
                out[name] = funcs - keep_funcs
        return out

    bacc.get_activation_tables = patched
    bacc._deform_act_patch = True


def _erf_gelu(nc, sb, out_ap, x_ap, shape):
    """out = x * (1 + erf(x/sqrt(2)))  (0.5 folded into wproj downstream).

    A&S 7.1.25 3-term for erf(|x|/sqrt2) = 1 - poly(t)*exp(-x^2/2),
    t = 1/(1 + p*|x|/sqrt2). Sign handled without bit tricks:
    with u = poly*e in (0, 1]:  x*(1+erf(x)) = x*u + relu(2*x*(1-u)).
    """
    P, Nf = shape
    sq = sb.tile([P, Nf], F32, name="gelu_sq", tag="gelu_sq")
    nc.scalar.activation(sq[:], x_ap, AF.Square)
    e = sb.tile([P, Nf], F32, name="gelu_e", tag="gelu_e")
    nc.scalar.activation(e[:], sq[:], AF.Exp, scale=-0.5)
    ax = sb.tile([P, Nf], F32, name="gelu_ax", tag="gelu_ax")
    nc.vector.scalar_tensor_tensor(ax[:], x_ap, -1.0, x_ap, ALU.mult, ALU.max)
    t = sb.tile([P, Nf], F32, name="gelu_t", tag="gelu_t")
    nc.vector.tensor_scalar(t[:], ax[:], float(ERF_P3 / np.sqrt(2.0)), 1.0, ALU.mult, ALU.add)
    nc.vector.reciprocal(t[:], t[:])
    poly = sb.tile([P, Nf], F32, name="gelu_poly", tag="gelu_poly")
    # poly = ((a3 t + a2) t + a1) t
    nc.vector.tensor_scalar(poly[:], t[:], ERF_A3[2], ERF_A3[1], ALU.mult, ALU.add)
    nc.vector.tensor_tensor(poly[:], poly[:], t[:], ALU.mult)
    nc.vector.scalar_tensor_tensor(poly[:], poly[:], ERF_A3[0], t[:], ALU.add, ALU.mult)
    u = sb.tile([P, Nf], F32, name="gelu_u", tag="gelu_u")
    nc.vector.tensor_tensor(u[:], poly[:], e[:], ALU.mult)
    xu = sb.tile([P, Nf], F32, name="gelu_xu", tag="gelu_xu")
    nc.vector.tensor_tensor(xu[:], x_ap, u[:], ALU.mult)
    w1 = sb.tile([P, Nf], F32, name="gelu_w1", tag="gelu_w1")
    nc.vector.tensor_tensor(w1[:], x_ap, xu[:], ALU.subtract)
    r = sb.tile([P, Nf], F32, name="gelu_r", tag="gelu_r")
    nc.scalar.activation(r[:], w1[:], AF.Relu, scale=2.0)
    nc.vector.tensor_tensor(out_ap, xu[:], r[:], ALU.add)


def _tanh_rows(nc, sb, out_ap, x_ap, shape):
    """out = tanh(x) = sign(x) * (1 - 2/(exp(2*min(|x|,30))+1)) on small tiles."""
    P, Nf = shape
    ax = sb.tile([P, Nf], F32, name="th_ax", tag="th_ax")
    nc.vector.scalar_tensor_tensor(ax[:], x_ap, -1.0, x_ap, ALU.mult, ALU.max)
    nc.vector.tensor_scalar(ax[:], ax[:], 30.0, None, ALU.min)
    e = sb.tile([P, Nf], F32, name="th_e", tag="th_e")
    nc.scalar.activation(e[:], ax[:], AF.Exp, scale=2.0)
    nc.vector.tensor_scalar(e[:], e[:], 1.0, None, ALU.add)
    r = sb.tile([P, Nf], F32, name="th_r", tag="th_r")
    nc.vector.reciprocal(r[:], e[:])
    # tha = 1 - 2r
    nc.vector.tensor_scalar(r[:], r[:], -2.0, 1.0, ALU.mult, ALU.add)
    sgn = sb.tile([P, Nf], U32, name="th_sgn", tag="th_sgn")
    nc.vector.tensor_scalar(sgn[:], x_ap.bitcast(U32), 0x80000000, None, ALU.bitwise_and)
    nc.vector.tensor_tensor(out_ap.bitcast(U32), r[:].bitcast(U32), sgn[:], ALU.bitwise_or)


# packed-weights column layout ([128, PCK] f32)
PK_WQTL = 0         # [wq[g].T | 0]         [64, 128]
PK_WQTH = 128       # [0 | wq[g].T]         [64, 128]
PK_WKT = 256        # wk[g].T               [64, 64]
PK_WVT = 320        # wv[g].T               [64, 64]
PK_WOT = 384        # wo[:, group cols].T   [64, 256]
PK_WQTS = 640       # wq[g].T * DH^-0.5     [64, 64]
PK_WEND = 704       # end of 64-row fp32r-copied region
PK_WDW = 704        # depthwise conv taps, rows duplicated  [128, 6]
PK_BDW = 710        # conv bias, duplicated                 [128, 1]
PK_WPJ = 711        # 0.5*w_off_proj block-diag cols        [128, 2]
PK_EYE = 713        # identity                              [64, 64]
PCK = 784


def build_nc():
    _patch_act_tables()
    nc = bacc.Bacc("TRN2", target_bir_lowering=False, debug=False, num_devices=NCORES)

    # ---- per-core DRAM inputs ----
    din = {}

    def dt_in(name, shape, dt=F32):
        din[name] = nc.dram_tensor(name, shape, dt, kind="ExternalInput")
        return din[name]

    dt_in("xg", [DPG, N], F32R)
    dt_in("xq", [DPG, QS], F32R)
    dt_in("xgT", [128, NT * DPG], F32R)
    dt_in("packed", [128, PCK])
    dt_in("slab", [128, 2 * NTT * QS], F32R)
    y_out = nc.dram_tensor("y", [DIM, QS], F32, kind="ExternalOutput")
    dbg = {}
    for nm, shp in [("dbg_qpad", [128, 514]), ("dbg_vbc", [1, NDS]),
                    ("dbg_kv", [DPG, NDS]), ("dbg_k", [DPG, NDS]),
                    ("dbg_qs", [DPG, QS]), ("dbg_avn", [DPG, QS]),
                    ("dbg_gl", [128, 128]), ("dbg_oht", [128, NTT * NDS]),
                    ("dbg_et", [128, QS])]:
        dbg[nm] = nc.dram_tensor(nm, shp, F32, kind="ExternalOutput")

    with tile.TileContext(nc) as tc:
        with (
            tc.tile_pool(name="const", bufs=1) as cst,
            tc.tile_pool(name="work", bufs=2) as wk,
            tc.tile_pool(name="rows", bufs=1) as rw,
            tc.tile_pool(name="persist", bufs=1) as pe_pool,
        ):
            # ---- input DMAs; slab split across both HWDGE queues ----
            packed = cst.tile([128, PCK], F32, name="packed", tag="packed")
            nc.sync.dma_start(packed[:], din["packed"].ap())
            xg = cst.tile([DPG, N], F32R, name="xg", tag="xg")
            nc.sync.dma_start(xg[:], din["xg"].ap())
            xq = cst.tile([DPG, QS], F32R, name="xq", tag="xq")
            nc.sync.dma_start(xq[:], din["xq"].ap())
            slabs = {}
            for t in range(NTT):
                for h in range(2):
                    sl = cst.tile([128, QS], F32R, name=f"slab{h}_{t}", tag=f"slab{h}_{t}")
                    slabs[(h, t)] = sl
                    q = nc.sync if h == 0 else nc.scalar
                    q.dma_start(sl[:], din["slab"].ap()[:, (h * NTT + t) * QS:(h * NTT + t + 1) * QS])
            xgT = cst.tile([128, NT * DPG], F32R, name="xgT", tag="xgT")
            nc.scalar.dma_start(xgT[:], din["xgT"].ap())

            # fp32r copies of the 64-row weight block + wproj columns
            wts = cst.tile([DPG, PK_WEND], F32R, name="wts", tag="wts")
            nc.vector.tensor_copy(wts[:], packed[0:DPG, 0:PK_WEND])
            wpj2r = cst.tile([128, 2], F32R, name="wpj2r", tag="wpj2r")
            nc.vector.tensor_copy(wpj2r[:], packed[:, PK_WPJ:PK_WPJ + 2])

            def W(col, width):
                return wts[:, col:col + width]

            eye64 = packed[0:DPG, PK_EYE:PK_EYE + DPG]

            ones_col = cst.tile([128, 1], F32, name="ones", tag="ones")
            nc.gpsimd.memset(ones_col[:], 1.0)
            ones32 = cst.tile([1, 32], F32, name="ones32", tag="ones32")
            nc.gpsimd.memset(ones32[:], 1.0)
            ones32r = cst.tile([1, 32], F32R, name="ones32r", tag="ones32r")
            nc.vector.tensor_copy(ones32r[:], ones32[:])
            ones_colr = cst.tile([128, 1], F32R, name="onesr", tag="onesr")
            nc.vector.tensor_copy(ones_colr[:], ones_col[:])
            # dummy activation: triggers the (single) ACT table load at t=0 so
            # it overlaps the input DMAs instead of sitting in the offsets chain
            warm = cst.tile([128, 1], F32, name="warm", tag="warm")
            nc.scalar.activation(warm[:], ones_col[:], AF.Relu)
            wz = cst.tile([128, 256], F32, name="wz", tag="wz")
            nc.gpsimd.memset(wz[:], 0.0)

            # iota-derived per-tile columns for the tent builds
            iotaS = cst.tile([128, NT], I32, name="iotaS", tag="iotaS")
            nc.gpsimd.iota(iotaS[:], pattern=[[128, NT]], base=0, channel_multiplier=1)
            iotaSf = cst.tile([128, NT], F32, name="iotaSf", tag="iotaSf")
            nc.vector.tensor_copy(iotaSf[:], iotaS[:])
            nc.vector.tensor_scalar(iotaSf[:], iotaSf[:], 0.5, None, ALU.add)
            iotaO = cst.tile([128, NTT], I32, name="iotaO", tag="iotaO")
            nc.gpsimd.iota(iotaO[:], pattern=[[128, NTT]], base=0, channel_multiplier=1)
            iotaOf = cst.tile([128, NTT], F32, name="iotaOf", tag="iotaOf")
            nc.vector.tensor_copy(iotaOf[:], iotaO[:])
            nc.vector.tensor_scalar(iotaOf[:], iotaOf[:], -float(CQ), None, ALU.add)

            # tent matrices (zeroed once; only static j-windows written later)
            S_all = pe_pool.tile([128, NT * NDS], F32R, name="S_all", tag="S_all")
            nc.gpsimd.memset(S_all[:].bitcast(F32), 0.0)
            OHT_all = pe_pool.tile([128, NTT * NDS], F32R, name="OHT_all", tag="OHT_all")
            nc.gpsimd.memset(OHT_all[:].bitcast(F32), 0.0)

            # persistent SBUF tiles that cross phase boundaries
            k_sb = pe_pool.tile([DPG, NDS], F32R, name="k_sb", tag="k_sb")
            qs_sb = pe_pool.tile([DPG, QS], F32R, name="qs_sb", tag="qs_sb")
            vT = [pe_pool.tile([128, DPG], F32R, name=f"vT{H}", tag=f"vT{H}") for H in range(2)]
            avn = pe_pool.tile([DPG, QS], F32R, name="avn", tag="avn")
            q_pad = pe_pool.tile([128, 514], F32, name="q_pad", tag="q_pad")
            v_bc = pe_pool.tile([128, NDS], F32, name="v_bc", tag="v_bc")

            # ============ phase A: q, qs, offsets ============
            with (
                tc.tile_pool(name="psA", bufs=2, space="PSUM") as psA,
                tc.tile_pool(name="psA1", bufs=1, space="PSUM") as psA1,
                tc.tile_pool(name="psW", bufs=1, space="PSUM") as psW,
            ):
                def warm_pe(n, tag):
                    for i in range(n):
                        wp = psW.tile([1, 256], F32, name=f"wp_{tag}{i}", tag="wp")
                        nc.tensor.matmul(wp[:], ones_col[:], wz[:])

                warm_pe(WARM_A, "a")

                # q in conv-packed layout: q_pad[c + 64b, 1+m] = q[c, 512b+m]
                nc.gpsimd.memset(q_pad[0:DPG, 0:1], 0.0)
                nc.gpsimd.memset(q_pad[DPG:128, 513:514], 0.0)
                pq2 = psA.tile([128, QS], F32, name="pq2", tag="pA512")
                nc.tensor.matmul(pq2[:], W(PK_WQTL, 128), xg[:, 0:QS],
                                 start=True, stop=False)
                nc.tensor.matmul(pq2[:], W(PK_WQTH, 128), xg[:, QS:N],
                                 start=False, stop=True)
                # boundary columns: q[:,512] for block 0, q[:,511] for block 1
                fix0 = psA1.tile([128, 2], F32, name="fix0", tag="small")
                nc.tensor.matmul(fix0[:], W(PK_WQTL, 128), xg[:, 512:514])
                fix1 = psA1.tile([128, 2], F32, name="fix1", tag="small2")
                nc.tensor.matmul(fix1[:], W(PK_WQTH, 128), xg[:, 510:512])
                # qs for this core's query half (scale folded in wqTs)
                pqs = psA.tile([DPG, QS], F32, name="pqs", tag="pA512")
                nc.tensor.matmul(pqs[:], W(PK_WQTS, DPG), xq[:])

                warm_pe(WARM_B, "b")

                nc.scalar.copy(q_pad[:, 1:513], pq2[:])
                nc.scalar.copy(q_pad[0:DPG, 513:514], fix0[0:DPG, 0:1])
                nc.scalar.copy(q_pad[DPG:128, 0:1], fix1[DPG:128, 1:2])
                nc.scalar.copy(qs_sb[:], pqs[:])

                nc.sync.dma_start(dbg["dbg_qpad"].ap(), q_pad[:])
                # depthwise strided conv (6 taps) at [128, 128]
                acc = wk.tile([128, 128], F32, name="conv_acc", tag="conv_acc")
                nc.vector.tensor_scalar(
                    acc[:], q_pad[:, 0:509:DS], packed[:, PK_WDW:PK_WDW + 1],
                    packed[:, PK_BDW:PK_BDW + 1], ALU.mult, ALU.add)
                for kk in range(1, OFF_K):
                    nc.vector.scalar_tensor_tensor(
                        acc[:], q_pad[:, kk:kk + 509:DS],
                        packed[:, PK_WDW + kk:PK_WDW + kk + 1], acc[:],
                        ALU.mult, ALU.add)

                gl = wk.tile([128, 128], F32R, name="gelu_out", tag="gelu_out")
                _erf_gelu(nc, wk, gl[:], acc[:], [128, 128])

                # proj halves as two single-row matmuls (both at partition 0)
                pproj = psA1.tile([1, 128], F32, name="pproj", tag="small")
                nc.tensor.matmul(pproj[:], wpj2r[:, 0:1], gl[:])
                pproj1 = psA1.tile([1, 128], F32, name="pproj1", tag="small2")
                nc.tensor.matmul(pproj1[:], wpj2r[:, 1:2], gl[:])

                warm_pe(WARM_C, "c")

                proj_sb = rw.tile([1, NDS], F32, name="proj_sb", tag="proj_sb")
                nc.vector.tensor_copy(proj_sb[:, 0:128], pproj[:])
                nc.scalar.copy(proj_sb[:, 128:256], pproj1[:])
                th = rw.tile([1, NDS], F32, name="th", tag="th")
                _tanh_rows(nc, rw, th[:], proj_sb[:], [1, NDS])

                # vg_raw = j + 4*tanh (scales folded into the tent builds)
                iotaj = rw.tile([1, NDS], I32, name="iotaj", tag="iotaj")
                nc.gpsimd.iota(iotaj[:], pattern=[[1, NDS]], base=0, channel_multiplier=0)
                iotajf = rw.tile([1, NDS], F32, name="iotajf", tag="iotajf")
                nc.vector.tensor_copy(iotajf[:], iotaj[:])
                vg = rw.tile([1, NDS], F32, name="vg", tag="vg")
                nc.vector.scalar_tensor_tensor(vg[:], th[:], OFF_SCALE, iotajf[:], ALU.mult, ALU.add)
                nc.gpsimd.partition_broadcast(v_bc[:], vg[:])

                nc.sync.dma_start(dbg["dbg_vbc"].ap(), v_bc[0:1, :])
                nc.sync.dma_start(dbg["dbg_gl"].ap(), gl[:].bitcast(F32))
                # ---- bias tents first (they gate the phase-D matmuls) ----
                for t in range(NTT):
                    jlo, jhi = _o_window(t)
                    if jhi <= jlo:
                        continue
                    w = jhi - jlo
                    wT = wk.tile([128, 48], F32, name="wT", tag="wT")
                    # w = c - kappa = (-A_O*vg) - (kappa - CQ)
                    nc.vector.tensor_scalar(
                        wT[:, 0:w], v_bc[:, jlo:jhi], -A_O, iotaOf[:, t:t + 1],
                        ALU.mult, ALU.subtract)
                    nc.vector.scalar_tensor_tensor(
                        wT[:, 0:w], wT[:, 0:w], -1.0, wT[:, 0:w], ALU.mult, ALU.min)
                    nc.scalar.activation(
                        OHT_all[:, NDS * t + jlo:NDS * t + jhi], wT[:, 0:w], AF.Relu, bias=1.0)

                # ---- kv-gather tents ----
                for t in range(NT):
                    jlo, jhi = _s_window(t)
                    w = jhi - jlo
                    wS = wk.tile([128, 48], F32, name="wS", tag="wS")
                    # w = ppix - pos = (A_S*vg) - (pos + 0.5)
                    nc.vector.tensor_scalar(
                        wS[:, 0:w], v_bc[:, jlo:jhi], A_S, iotaSf[:, t:t + 1],
                        ALU.mult, ALU.subtract)
                    nc.vector.scalar_tensor_tensor(
                        wS[:, 0:w], wS[:, 0:w], -1.0, wS[:, 0:w], ALU.mult, ALU.min)
                    nc.scalar.activation(
                        S_all[:, NDS * t + jlo:NDS * t + jhi], wS[:, 0:w], AF.Relu, bias=1.0)

                # kv gather + k, v, vT
                pkv = psA.tile([DPG, NDS], F32, name="pA256", tag="pA256")
                for t in range(NT):
                    nc.tensor.matmul(pkv[:], xgT[:, DPG * t:DPG * (t + 1)],
                                     S_all[:, NDS * t:NDS * (t + 1)],
                                     start=(t == 0), stop=(t == NT - 1))
                kv = wk.tile([DPG, NDS], F32R, name="kv", tag="kv")
                nc.scalar.copy(kv[:], pkv[:])
                nc.sync.dma_start(dbg["dbg_kv"].ap(), kv[:].bitcast(F32))
                nc.sync.dma_start(dbg["dbg_qs"].ap(), qs_sb[:].bitcast(F32))
                nc.sync.dma_start(dbg["dbg_oht"].ap(), OHT_all[:].bitcast(F32))

                pk = psA.tile([DPG, NDS], F32, name="pA256", tag="pA256")
                nc.tensor.matmul(pk[:], W(PK_WKT, DPG), kv[:])
                nc.scalar.copy(k_sb[:], pk[:])
                nc.sync.dma_start(dbg["dbg_k"].ap(), k_sb[:].bitcast(F32))
                pv = psA.tile([DPG, NDS], F32, name="pA256", tag="pA256")
                nc.tensor.matmul(pv[:], W(PK_WVT, DPG), kv[:])
                v_sb = wk.tile([DPG, NDS], F32, name="v_sb", tag="v_sb")
                nc.vector.tensor_copy(v_sb[:], pv[:])

                for H in range(2):
                    pt = psA1.tile([128, DPG], F32, name="ptp", tag="ptp")
                    nc.tensor.transpose(pt[:], v_sb[:, H * 128:(H + 1) * 128], eye64)
                    nc.scalar.copy(vT[H][:], pt[:])

            # ============ phase D: attention (bias accumulated in PSUM) ============
            with (
                tc.tile_pool(name="psE", bufs=1, space="PSUM") as psE,
                tc.tile_pool(name="psE1", bufs=1, space="PSUM") as psE1,
            ):
                psims = {}
                for h in range(2):
                    for H in range(2):
                        psims[(h, H)] = psE.tile([128, QS], F32, name=f"psim{h}{H}", tag=f"psim{h}{H}")
                # per-group: 9 bias matmuls (tent x Hankel slab) + q.k close
                expT = {}
                for h in range(2):
                    for H in range(2):
                        for t in range(NTT):
                            nc.tensor.matmul(
                                psims[(h, H)][:],
                                OHT_all[:, NDS * t + 128 * H:NDS * t + 128 * (H + 1)],
                                slabs[(h, t)][:],
                                start=(t == 0), stop=False)
                        nc.tensor.matmul(
                            psims[(h, H)][:], k_sb[32 * h:32 * (h + 1), H * 128:(H + 1) * 128],
                            qs_sb[32 * h:32 * (h + 1), :], start=False, stop=True)
                        et = wk.tile([128, QS], F32R, name=f"expT{h}{H}", tag=f"expT{h}{H}")
                        nc.scalar.activation(et[:], psims[(h, H)][:], AF.Exp)
                        expT[(h, H)] = et

                for h in range(2):
                    psum_s = psE1.tile([1, QS], F32, name="psum_s", tag="psum_s")
                    for H in range(2):
                        nc.tensor.matmul(psum_s[:], ones_colr[:], expT[(h, H)][:],
                                         start=(H == 0), stop=(H == 1))
                    rs = rw.tile([1, QS], F32R, name="rs", tag="rs")
                    with nc.allow_low_precision(reason="f32r bits == f32; PE rounds anyway"):
                        nc.vector.reciprocal(rs[:], psum_s[:])
                    # broadcast 1/s to 32 partitions via PE (ones32^T @ rs)
                    rsb = psE1.tile([32, QS], F32, name="rsb", tag="rsb")
                    nc.tensor.matmul(rsb[:], ones32r[:], rs[:])

                    pav = psE1.tile([32, QS], F32, name="pav", tag="pav")
                    for H in range(2):
                        nc.tensor.matmul(pav[:], vT[H][:, 32 * h:32 * (h + 1)], expT[(h, H)][:],
                                         start=(H == 0), stop=(H == 1))
                    pav_sb = wk.tile([32, QS], F32, name="pav_sb", tag="pav_sb")
                    nc.scalar.copy(pav_sb[:], pav[:])
                    nc.vector.tensor_tensor(avn[32 * h:32 * (h + 1), :], pav_sb[:], rsb[:], ALU.mult)

                nc.sync.dma_start(dbg["dbg_avn"].ap(), avn[:].bitcast(F32))
                nc.sync.dma_start(dbg["dbg_et"].ap(), expT[(0, 0)][:].bitcast(F32))
                for m in range(2):
                    py = psE.tile([128, QS], F32, name="py", tag=f"psim0{m}")
                    nc.tensor.matmul(py[:], W(PK_WOT + m * 128, 128), avn[:])
                    y_sb = wk.tile([128, QS], F32, name="y_sb", tag="y_sb")
                    if m == 0:
                        nc.vector.tensor_copy(y_sb[:], py[:])
                        nc.sync.dma_start(y_out.ap()[m * 128:(m + 1) * 128, :], y_sb[:])
                    else:
                        nc.scalar.copy(y_sb[:], py[:])
                        nc.scalar.dma_start(y_out.ap()[m * 128:(m + 1) * 128, :], y_sb[:])

    nc.compile()
    return nc


def _shard_inputs(inputs):
    """Build the 8 per-core input maps from the full inputs."""
    x = np.ascontiguousarray(inputs["x"][0])               # [256, 1024]
    wq, wk, wv = inputs["wq"], inputs["wk"], inputs["wv"]  # [4, 64, 64]
    wo = inputs["wo"]                                      # [256, 256]
    w_off_dw = inputs["w_off_dw"][:, 0, :]                 # [64, 6]
    b_off_dw = inputs["b_off_dw"]                          # [64]
    w_off_proj = inputs["w_off_proj"]                      # [64]
    w1 = inputs["cpb_w1"][:, 0].astype(np.float64)         # [64]
    b1 = inputs["cpb_b1"].astype(np.float64)
    w2 = inputs["cpb_w2"].astype(np.float64)
    b2 = inputs["cpb_b2"].astype(np.float64)
    w3 = inputs["cpb_w3"].astype(np.float64)               # [2, 64]
    b3 = inputs["cpb_b3"].astype(np.float64)

    f = np.float32

    # bias lookup tables + Hankel slabs, one per query-half (weights-only)
    slabs = {}
    for qh in range(2):
        kk = np.arange(LTAB, dtype=np.float64)
        d = HQ * (kk - CQ + QS * qh)
        pos = np.sign(d) * np.log1p(np.abs(d))
        h1 = np.maximum(pos[:, None] * w1[None, :] + b1, 0.0)
        h2 = np.maximum(h1 @ w2.T + b2, 0.0)
        T = (h2 @ w3.T + b3).astype(f)                     # [LTAB, 2]
        sl = np.zeros((128, 2 * NTT * QS), f)
        for o in range(2):
            sw = np.lib.stride_tricks.sliding_window_view(T[:, o], QS)
            for t in range(NTT):
                sl[:, (o * NTT + t) * QS:(o * NTT + t + 1) * QS] = sw[128 * t:128 * t + 128]
        slabs[qh] = sl

    base_packed = np.zeros((128, PCK), f)
    base_packed[:, PK_WDW:PK_WDW + OFF_K] = np.concatenate([w_off_dw, w_off_dw], 0)
    base_packed[:, PK_BDW] = np.concatenate([b_off_dw, b_off_dw], 0)
    base_packed[0:DPG, PK_WPJ] = 0.5 * w_off_proj
    base_packed[DPG:128, PK_WPJ + 1] = 0.5 * w_off_proj
    base_packed[0:DPG, PK_EYE:PK_EYE + DPG] = np.eye(DPG, dtype=f)

    in_maps = []
    for c in range(NCORES):
        g, qh = c // 2, c % 2
        xg = np.ascontiguousarray(x[DPG * g:DPG * (g + 1)], dtype=f)
        xgT = np.zeros((128, NT * DPG), f)
        for t in range(NT):
            xgT[:, DPG * t:DPG * (t + 1)] = xg[:, 128 * t:128 * (t + 1)].T
        pk = base_packed.copy()
        pk[0:DPG, PK_WQTL:PK_WQTL + DPG] = wq[g].T
        pk[0:DPG, PK_WQTH + DPG:PK_WQTH + 128] = wq[g].T
        pk[0:DPG, PK_WKT:PK_WKT + DPG] = wk[g].T
        pk[0:DPG, PK_WVT:PK_WVT + DPG] = wv[g].T
        pk[0:DPG, PK_WOT:PK_WOT + DIM] = wo[:, DPG * g:DPG * (g + 1)].T
        pk[0:DPG, PK_WQTS:PK_WQTS + DPG] = wq[g].T * f(DH) ** f(-0.5)
        m = {
            "xg": xg,
            "xq": np.ascontiguousarray(xg[:, QS * qh:QS * (qh + 1)]),
            "xgT": xgT,
            "packed": pk,
            "slab": slabs[qh],
        }
        in_maps.append(m)
    return in_maps


def kernel(**inputs):
    if "nc" not in _CACHED:
        _CACHED["nc"] = build_nc()
    nc = _CACHED["nc"]
    in_maps = _shard_inputs(inputs)
    res = bass_utils.run_bass_kernel_spmd(nc, in_maps, core_ids=list(range(NCORES)))
    ys = [res.results[c]["y"] for c in range(NCORES)]
    bo = inputs["bo"]
    out = np.zeros((1, DIM, N), np.float32)
    for qh in range(2):
        acc = np.zeros((DIM, QS), np.float64)
        for g in range(G):
            acc += ys[2 * g + qh]
        out[0, :, QS * qh:QS * (qh + 1)] = (acc + bo.astype(np.float64)[:, None]).astype(np.float32)
    return out


# revision 21
# speedup vs baseline: 1.1421x; 1.1421x over previous
"""DeformableAttention1D on 8 TRN2 NeuronCores via Bass/Tile.

Sharding: core c handles offset-group g=c//2 (64 of 256 channels, 2 of 8 heads)
and query-half qh=c%2 (512 of 1024 positions). Each core computes its group's
offsets/gather/bias/attention independently; the final output projection is
computed as a partial (wo sliced by group) and summed on the host (the
"all-reduce" of the output projection).

Key idea vs the straightforward implementation: the CPB relative-position-bias
MLP is a scalar->2 function F(d) of the signed distance d = gq_i - vgsp1_j,
and the query grid steps uniformly by h_q = 2/1023. Tabulating F on a grid
with spacing exactly h_q (a host-side weights-only precompute, like any other
weight repacking) turns the bias into

    bias[i, j] = (1-r_j) T[m_j + i] + r_j T[m_j + 1 + i],   m_j + r_j = c_j,

i.e. a per-column shifted window of the table. On device this is 9 matmuls of
"tent" interpolation one-hots max(0, 1-|c_j - kappa|) (stationary) against
host-precomputed Hankel slabs of T (moving), accumulated directly into the
attention-logit PSUM on top of q.k — the bias costs zero vector-engine work
in the attention phase. The tent matrices also implement the bilinear kv
grid-sample gather (zeros padding included). Tents are only materialized in
their static nonzero j-windows (~43 of 256 columns per kappa-tile).

Device numerics: fp32 data, fp32r matmuls (1 cycle/col vs 4 for fp32). The
ACT engine is restricted to ONE table set (natural_log_exp_and_others:
Exp/Ln/Relu/Copy/Identity/Square); tanh and erf(gelu) are composed from
Exp + DVE ops. Max bias interp error ~3e-4 in the logits.
"""
import os
import sys

sys.path.insert(0, "/opt/trn_rl_repo")

import numpy as np

import concourse.bacc as bacc
import concourse.bass as bass
import concourse.mybir as mybir
import concourse.tile as tile
import concourse.bass_utils as bass_utils

F32 = mybir.dt.float32
F32R = mybir.dt.float32r
I32 = mybir.dt.int32
U32 = mybir.dt.uint32
AF = mybir.ActivationFunctionType
ALU = mybir.AluOpType

# model dims (hardcoded per problem spec)
DIM = 256
N = 1024
G = 4
HEADS = 8
DH = 32
NDS = 256          # downsampled kv positions
QS = 512           # queries per core
DPG = 64           # channels per group
OFF_K = 6
DS = 4             # downsample stride
OFF_SCALE = 4.0
NCORES = 8

# bias lookup table
HQ = 2.0 / 1023.0  # query grid step == table spacing
CQ = 1040.0        # index offset so c_j = CQ - vgsp1_j/HQ stays in [0.9, 1057]
LTAB = 1664        # table length (slabs need up to 128*8+127+511 = 1662)
NTT = 9            # tent kappa-tiles (c_j+1 < 1152)
NT = 8             # x position tiles for the kv gather
A_S = 1024.0 / 255.0   # d ppix / d vg_raw  (|offset| < 4 -> +-4*A_S slack)
A_O = 1023.0 / 255.0   # -d c / d vg_raw

# A&S 7.1.25 3-term erf coefficients (|err| <= 2.5e-5)
ERF_P3 = 0.47047
ERF_A3 = [0.3480242, -0.0958798, 0.7478556]

# PE warmup chain lengths (tuned against the cost-model timeline)
WARM_A = 4
WARM_B = 12
WARM_C = 7

_CACHED = {}


def _s_window(t):
    """Static j-range where the kv-gather tent for position tile t can be
    nonzero: ppix_j in (128t-1, 128t+128), ppix = A_S*vg_raw-0.5, vg_raw in
    (j-4, j+4)."""
    jlo = max(0, int(np.floor((128 * t - 0.5) / A_S - 4)) - 1)
    jhi = min(NDS, int(np.ceil((128 * t + 128.5) / A_S + 4)) + 1)
    return jlo, jhi


def _o_window(t):
    """Static j-range where the bias tent for kappa tile t can be nonzero:
    c_j in (128t-1, 128t+128), c = CQ - A_O*vg_raw."""
    jlo = max(0, int(np.floor((912 - 128 * t) / A_O - 4)) - 1)
    jhi = min(NDS, int(np.ceil((1041 - 128 * t) / A_O + 4)) + 1)
    return jlo, jhi


def _patch_act_tables():
    """Restrict activation-table selection to the single set that covers all
    ACT functions used by this kernel, so exactly one table load is emitted
    (runtime table swaps do not work in this environment)."""
    import concourse.hw_specs as hw_specs

    if getattr(bacc, "_deform_act_patch", False):
        return
    orig = hw_specs.get_activation_tables

    keep = "natural_log_exp_and_others"

    def patched(module_arch):
        tabs = orig(module_arch)
        keep_funcs = tabs[keep]
        out = {}
        for name, funcs in tabs.items():
            if name == keep:
                out[name] = funcs
            else:
                out[name] = funcs - keep_funcs
        return out

    bacc.get_activation_tables = patched
    bacc._deform_act_patch = True


def _erf_gelu(nc, sb, out_ap, x_ap, shape):
    """out = x * (1 + erf(x/sqrt(2)))  (0.5 folded into wproj downstream).

    A&S 7.1.25 3-term for erf(|x|/sqrt2) = 1 - poly(t)*exp(-x^2/2),
    t = 1/(1 + p*|x|/sqrt2). Sign handled without bit tricks:
    with u = poly*e in (0, 1]:  x*(1+erf(x)) = x*u + relu(2*x*(1-u)).
    """
    P, Nf = shape
    sq = sb.tile([P, Nf], F32, name="gelu_sq", tag="gelu_sq")
    nc.scalar.activation(sq[:], x_ap, AF.Square)
    e = sb.tile([P, Nf], F32, name="gelu_e", tag="gelu_e")
    nc.scalar.activation(e[:], sq[:], AF.Exp, scale=-0.5)
    ax = sb.tile([P, Nf], F32, name="gelu_ax", tag="gelu_ax")
    nc.vector.scalar_tensor_tensor(ax[:], x_ap, -1.0, x_ap, ALU.mult, ALU.max)
    t = sb.tile([P, Nf], F32, name="gelu_t", tag="gelu_t")
    nc.vector.tensor_scalar(t[:], ax[:], float(ERF_P3 / np.sqrt(2.0)), 1.0, ALU.mult, ALU.add)
    nc.vector.reciprocal(t[:], t[:])
    poly = sb.tile([P, Nf], F32, name="gelu_poly", tag="gelu_poly")
    # poly = ((a3 t + a2) t + a1) t
    nc.vector.tensor_scalar(poly[:], t[:], ERF_A3[2], ERF_A3[1], ALU.mult, ALU.add)
    nc.vector.tensor_tensor(poly[:], poly[:], t[:], ALU.mult)
    nc.vector.scalar_tensor_tensor(poly[:], poly[:], ERF_A3[0], t[:], ALU.add, ALU.mult)
    u = sb.tile([P, Nf], F32, name="gelu_u", tag="gelu_u")
    nc.vector.tensor_tensor(u[:], poly[:], e[:], ALU.mult)
    xu = sb.tile([P, Nf], F32, name="gelu_xu", tag="gelu_xu")
    nc.vector.tensor_tensor(xu[:], x_ap, u[:], ALU.mult)
    w1 = sb.tile([P, Nf], F32, name="gelu_w1", tag="gelu_w1")
    nc.vector.tensor_tensor(w1[:], x_ap, xu[:], ALU.subtract)
    r = sb.tile([P, Nf], F32, name="gelu_r", tag="gelu_r")
    nc.scalar.activation(r[:], w1[:], AF.Relu, scale=2.0)
    nc.vector.tensor_tensor(out_ap, xu[:], r[:], ALU.add)


def _tanh_rows(nc, sb, out_ap, x_ap, shape):
    """out = tanh(x) = sign(x) * (1 - 2/(exp(2*min(|x|,30))+1)) on small tiles."""
    P, Nf = shape
    ax = sb.tile([P, Nf], F32, name="th_ax", tag="th_ax")
    nc.vector.scalar_tensor_tensor(ax[:], x_ap, -1.0, x_ap, ALU.mult, ALU.max)
    nc.vector.tensor_scalar(ax[:], ax[:], 30.0, None, ALU.min)
    e = sb.tile([P, Nf], F32, name="th_e", tag="th_e")
    nc.scalar.activation(e[:], ax[:], AF.Exp, scale=2.0)
    nc.vector.tensor_scalar(e[:], e[:], 1.0, None, ALU.add)
    r = sb.tile([P, Nf], F32, name="th_r", tag="th_r")
    nc.vector.reciprocal(r[:], e[:])
    # tha = 1 - 2r
    nc.vector.tensor_scalar(r[:], r[:], -2.0, 1.0, ALU.mult, ALU.add)
    sgn = sb.tile([P, Nf], U32, name="th_sgn", tag="th_sgn")
    nc.vector.tensor_scalar(sgn[:], x_ap.bitcast(U32), 0x80000000, None, ALU.bitwise_and)
    nc.vector.tensor_tensor(out_ap.bitcast(U32), r[:].bitcast(U32), sgn[:], ALU.bitwise_or)


# packed-weights column layout ([128, PCK] f32)
PK_WQTL = 0         # [wq[g].T | 0]         [64, 128]
PK_WQTH = 128       # [0 | wq[g].T]         [64, 128]
PK_WKT = 256        # wk[g].T               [64, 64]
PK_WVT = 320        # wv[g].T               [64, 64]
PK_WOT = 384        # wo[:, group cols].T   [64, 256]
PK_WQTS = 640       # wq[g].T * DH^-0.5     [64, 64]
PK_WEND = 704       # end of 64-row fp32r-copied region
PK_WDW = 704        # depthwise conv taps, rows duplicated  [128, 6]
PK_BDW = 710        # conv bias, duplicated                 [128, 1]
PK_WPJ = 711        # 0.5*w_off_proj block-diag cols        [128, 2]
PK_EYE = 713        # identity                              [64, 64]
PCK = 784


def build_nc():
    _patch_act_tables()
    nc = bacc.Bacc("TRN2", target_bir_lowering=False, debug=False, num_devices=NCORES)

    # ---- per-core DRAM inputs ----
    din = {}

    def dt_in(name, shape, dt=F32):
        din[name] = nc.dram_tensor(name, shape, dt, kind="ExternalInput")
        return din[name]

    dt_in("xg", [DPG, N], F32R)
    dt_in("xq", [DPG, QS], F32R)
    dt_in("xgT", [128, NT * DPG], F32R)
    dt_in("packed", [128, PCK])
    dt_in("hank", [128, 2 * 1536], F32R)
    y_out = nc.dram_tensor("y", [DIM, QS], F32, kind="ExternalOutput")

    with tile.TileContext(nc) as tc:
        with (
            tc.tile_pool(name="const", bufs=1) as cst,
            tc.tile_pool(name="work", bufs=2) as wk,
            tc.tile_pool(name="rows", bufs=1) as rw,
            tc.tile_pool(name="persist", bufs=1) as pe_pool,
        ):
            # ---- input DMAs; slab split across both HWDGE queues ----
            packed = cst.tile([128, PCK], F32, name="packed", tag="packed")
            nc.sync.dma_start(packed[:], din["packed"].ap())
            xg = cst.tile([DPG, N], F32R, name="xg", tag="xg")
            nc.sync.dma_start(xg[:], din["xg"].ap())
            xq = cst.tile([DPG, QS], F32R, name="xq", tag="xq")
            nc.sync.dma_start(xq[:], din["xq"].ap())
            xgT = cst.tile([128, NT * DPG], F32R, name="xgT", tag="xgT")
            nc.sync.dma_start(xgT[:], din["xgT"].ap())
            # Hankel strip HH[k', c] = T[k' + c]; the 9 Hankel "slabs" per head
            # are its sliding 512-col windows, so one DMA covers them all
            hank = cst.tile([128, 2 * 1536], F32R, name="hank", tag="hank")
            nc.sync.dma_start(hank[:], din["hank"].ap())

            # fp32r copies of the 64-row weight block + wproj columns
            wts = cst.tile([DPG, PK_WEND], F32R, name="wts", tag="wts")
            nc.vector.tensor_copy(wts[:], packed[0:DPG, 0:PK_WEND])
            wpj2r = cst.tile([128, 2], F32R, name="wpj2r", tag="wpj2r")
            nc.vector.tensor_copy(wpj2r[:], packed[:, PK_WPJ:PK_WPJ + 2])

            def W(col, width):
                return wts[:, col:col + width]

            eye64 = packed[0:DPG, PK_EYE:PK_EYE + DPG]

            ones_col = cst.tile([128, 1], F32, name="ones", tag="ones")
            nc.gpsimd.memset(ones_col[:], 1.0)
            ones32 = cst.tile([1, 32], F32, name="ones32", tag="ones32")
            nc.gpsimd.memset(ones32[:], 1.0)
            ones32r = cst.tile([1, 32], F32R, name="ones32r", tag="ones32r")
            nc.vector.tensor_copy(ones32r[:], ones32[:])
            ones_colr = cst.tile([128, 1], F32R, name="onesr", tag="onesr")
            nc.vector.tensor_copy(ones_colr[:], ones_col[:])
            # dummy activation: triggers the (single) ACT table load at t=0 so
            # it overlaps the input DMAs instead of sitting in the offsets chain
            warm = cst.tile([128, 1], F32, name="warm", tag="warm")
            nc.scalar.activation(warm[:], ones_col[:], AF.Relu)
            wz = cst.tile([128, 256], F32, name="wz", tag="wz")
            nc.gpsimd.memset(wz[:], 0.0)

            # iota-derived per-tile columns for the tent builds
            iotaS = cst.tile([128, NT], I32, name="iotaS", tag="iotaS")
            nc.gpsimd.iota(iotaS[:], pattern=[[128, NT]], base=0, channel_multiplier=1)
            iotaSf = cst.tile([128, NT], F32, name="iotaSf", tag="iotaSf")
            nc.vector.tensor_copy(iotaSf[:], iotaS[:])
            nc.vector.tensor_scalar(iotaSf[:], iotaSf[:], 0.5, None, ALU.add)
            iotaO = cst.tile([128, NTT], I32, name="iotaO", tag="iotaO")
            nc.gpsimd.iota(iotaO[:], pattern=[[128, NTT]], base=0, channel_multiplier=1)
            iotaOf = cst.tile([128, NTT], F32, name="iotaOf", tag="iotaOf")
            nc.vector.tensor_copy(iotaOf[:], iotaO[:])
            nc.vector.tensor_scalar(iotaOf[:], iotaOf[:], -float(CQ), None, ALU.add)

            # tent matrices (zeroed once; only static j-windows written later)
            S_all = pe_pool.tile([128, NT * NDS], F32R, name="S_all", tag="S_all")
            nc.gpsimd.memset(S_all[:].bitcast(F32), 0.0)
            OHT_all = pe_pool.tile([128, NTT * NDS], F32R, name="OHT_all", tag="OHT_all")
            nc.gpsimd.memset(OHT_all[:].bitcast(F32), 0.0)

            # persistent SBUF tiles that cross phase boundaries
            k_sb = pe_pool.tile([DPG, NDS], F32R, name="k_sb", tag="k_sb")
            qs_sb = pe_pool.tile([DPG, QS], F32R, name="qs_sb", tag="qs_sb")
            vT = [pe_pool.tile([128, DPG], F32R, name=f"vT{H}", tag=f"vT{H}") for H in range(2)]
            avn = pe_pool.tile([DPG, QS], F32R, name="avn", tag="avn")
            q_pad = pe_pool.tile([128, 514], F32, name="q_pad", tag="q_pad")
            v_bc = pe_pool.tile([128, NDS], F32, name="v_bc", tag="v_bc")

            # ============ phase A: q, qs, offsets ============
            with (
                tc.tile_pool(name="psA", bufs=2, space="PSUM") as psA,
                tc.tile_pool(name="psA1", bufs=1, space="PSUM") as psA1,
                tc.tile_pool(name="psW", bufs=1, space="PSUM") as psW,
            ):
                def warm_pe(n, tag):
                    for i in range(n):
                        wp = psW.tile([1, 256], F32, name=f"wp_{tag}{i}", tag="wp")
                        nc.tensor.matmul(wp[:], ones_col[:], wz[:])

                warm_pe(WARM_A, "a")

                # q in conv-packed layout: q_pad[c + 64b, 1+m] = q[c, 512b+m]
                nc.gpsimd.memset(q_pad[0:DPG, 0:1], 0.0)
                nc.gpsimd.memset(q_pad[DPG:128, 513:514], 0.0)
                pq2 = psA.tile([128, QS], F32, name="pq2", tag="pA512")
                nc.tensor.matmul(pq2[:], W(PK_WQTL, 128), xg[:, 0:QS],
                                 start=True, stop=False)
                nc.tensor.matmul(pq2[:], W(PK_WQTH, 128), xg[:, QS:N],
                                 start=False, stop=True)
                # boundary columns: q[:,512] for block 0, q[:,511] for block 1
                fix0 = psA1.tile([128, 2], F32, name="fix0", tag="small")
                nc.tensor.matmul(fix0[:], W(PK_WQTL, 128), xg[:, 512:514])
                fix1 = psA1.tile([128, 2], F32, name="fix1", tag="small2")
                nc.tensor.matmul(fix1[:], W(PK_WQTH, 128), xg[:, 510:512])
                # qs for this core's query half (scale folded in wqTs)
                pqs = psA.tile([DPG, QS], F32, name="pqs", tag="pA512")
                nc.tensor.matmul(pqs[:], W(PK_WQTS, DPG), xq[:])

                warm_pe(WARM_B, "b")

                nc.scalar.copy(q_pad[:, 1:513], pq2[:])
                nc.scalar.copy(q_pad[0:DPG, 513:514], fix0[0:DPG, 0:1])
                nc.scalar.copy(q_pad[DPG:128, 0:1], fix1[DPG:128, 1:2])
                nc.scalar.copy(qs_sb[:], pqs[:])

                # depthwise strided conv (6 taps) at [128, 128]
                acc = wk.tile([128, 128], F32, name="conv_acc", tag="conv_acc")
                nc.vector.tensor_scalar(
                    acc[:], q_pad[:, 0:509:DS], packed[:, PK_WDW:PK_WDW + 1],
                    packed[:, PK_BDW:PK_BDW + 1], ALU.mult, ALU.add)
                for kk in range(1, OFF_K):
                    nc.vector.scalar_tensor_tensor(
                        acc[:], q_pad[:, kk:kk + 509:DS],
                        packed[:, PK_WDW + kk:PK_WDW + kk + 1], acc[:],
                        ALU.mult, ALU.add)

                gl = wk.tile([128, 128], F32R, name="gelu_out", tag="gelu_out")
                _erf_gelu(nc, wk, gl[:], acc[:], [128, 128])

                # proj halves as two single-row matmuls (both at partition 0)
                pproj = psA1.tile([1, 128], F32, name="pproj", tag="small")
                nc.tensor.matmul(pproj[:], wpj2r[:, 0:1], gl[:])
                pproj1 = psA1.tile([1, 128], F32, name="pproj1", tag="small2")
                nc.tensor.matmul(pproj1[:], wpj2r[:, 1:2], gl[:])

                warm_pe(WARM_C, "c")

                proj_sb = rw.tile([1, NDS], F32, name="proj_sb", tag="proj_sb")
                nc.vector.tensor_copy(proj_sb[:, 0:128], pproj[:])
                nc.scalar.copy(proj_sb[:, 128:256], pproj1[:])
                th = rw.tile([1, NDS], F32, name="th", tag="th")
                _tanh_rows(nc, rw, th[:], proj_sb[:], [1, NDS])

                # vg_raw = j + 4*tanh (scales folded into the tent builds)
                iotaj = rw.tile([1, NDS], I32, name="iotaj", tag="iotaj")
                nc.gpsimd.iota(iotaj[:], pattern=[[1, NDS]], base=0, channel_multiplier=0)
                iotajf = rw.tile([1, NDS], F32, name="iotajf", tag="iotajf")
                nc.vector.tensor_copy(iotajf[:], iotaj[:])
                vg = rw.tile([1, NDS], F32, name="vg", tag="vg")
                nc.vector.scalar_tensor_tensor(vg[:], th[:], OFF_SCALE, iotajf[:], ALU.mult, ALU.add)
                nc.gpsimd.partition_broadcast(v_bc[:], vg[:])

                # ---- bias tents first (they gate the phase-D matmuls) ----
                for t in range(NTT):
                    jlo, jhi = _o_window(t)
                    if jhi <= jlo:
                        continue
                    w = jhi - jlo
                    wT = wk.tile([128, 48], F32, name="wT", tag="wT")
                    # w = c - kappa = (-A_O*vg) - (kappa - CQ)
                    nc.vector.tensor_scalar(
                        wT[:, 0:w], v_bc[:, jlo:jhi], -A_O, iotaOf[:, t:t + 1],
                        ALU.mult, ALU.subtract)
                    nc.vector.scalar_tensor_tensor(
                        wT[:, 0:w], wT[:, 0:w], -1.0, wT[:, 0:w], ALU.mult, ALU.min)
                    nc.scalar.activation(
                        OHT_all[:, NDS * t + jlo:NDS * t + jhi], wT[:, 0:w], AF.Relu, bias=1.0)

                # ---- kv-gather tents ----
                for t in range(NT):
                    jlo, jhi = _s_window(t)
                    w = jhi - jlo
                    wS = wk.tile([128, 48], F32, name="wS", tag="wS")
                    # w = ppix - pos = (A_S*vg) - (pos + 0.5)
                    nc.vector.tensor_scalar(
                        wS[:, 0:w], v_bc[:, jlo:jhi], A_S, iotaSf[:, t:t + 1],
                        ALU.mult, ALU.subtract)
                    nc.vector.scalar_tensor_tensor(
                        wS[:, 0:w], wS[:, 0:w], -1.0, wS[:, 0:w], ALU.mult, ALU.min)
                    nc.scalar.activation(
                        S_all[:, NDS * t + jlo:NDS * t + jhi], wS[:, 0:w], AF.Relu, bias=1.0)

                # kv gather + k, v, vT
                pkv = psA.tile([DPG, NDS], F32, name="pA256", tag="pA256")
                for t in range(NT):
                    nc.tensor.matmul(pkv[:], xgT[:, DPG * t:DPG * (t + 1)],
                                     S_all[:, NDS * t:NDS * (t + 1)],
                                     start=(t == 0), stop=(t == NT - 1))
                kv = wk.tile([DPG, NDS], F32R, name="kv", tag="kv")
                nc.scalar.copy(kv[:], pkv[:])

                pk = psA.tile([DPG, NDS], F32, name="pA256", tag="pA256")
                nc.tensor.matmul(pk[:], W(PK_WKT, DPG), kv[:])
                nc.scalar.copy(k_sb[:], pk[:])
                pv = psA.tile([DPG, NDS], F32, name="pA256", tag="pA256")
                nc.tensor.matmul(pv[:], W(PK_WVT, DPG), kv[:])
                v_sb = wk.tile([DPG, NDS], F32, name="v_sb", tag="v_sb")
                nc.vector.tensor_copy(v_sb[:], pv[:])

                for H in range(2):
                    pt = psA1.tile([128, DPG], F32, name="ptp", tag="ptp")
                    nc.tensor.transpose(pt[:], v_sb[:, H * 128:(H + 1) * 128], eye64)
                    nc.scalar.copy(vT[H][:], pt[:])

            # ============ phase D: attention (bias accumulated in PSUM) ============
            with (
                tc.tile_pool(name="psE", bufs=1, space="PSUM") as psE,
                tc.tile_pool(name="psE1", bufs=1, space="PSUM") as psE1,
            ):
                psims = {}
                for h in range(2):
                    for H in range(2):
                        psims[(h, H)] = psE.tile([128, QS], F32, name=f"psim{h}{H}", tag=f"psim{h}{H}")
                # per-group: 9 bias matmuls (tent x Hankel slab) + q.k close
                expT = {}
                for h in range(2):
                    for H in range(2):
                        for t in range(NTT):
                            nc.tensor.matmul(
                                psims[(h, H)][:],
                                OHT_all[:, NDS * t + 128 * H:NDS * t + 128 * (H + 1)],
                                hank[:, h * 1536 + 128 * t:h * 1536 + 128 * t + QS],
                                start=(t == 0), stop=False)
                        nc.tensor.matmul(
                            psims[(h, H)][:], k_sb[32 * h:32 * (h + 1), H * 128:(H + 1) * 128],
                            qs_sb[32 * h:32 * (h + 1), :], start=False, stop=True)
                        et = wk.tile([128, QS], F32R, name=f"expT{h}{H}", tag=f"expT{h}{H}")
                        nc.scalar.activation(et[:], psims[(h, H)][:], AF.Exp)
                        expT[(h, H)] = et

                for h in range(2):
                    psum_s = psE1.tile([1, QS], F32, name="psum_s", tag="psum_s")
                    for H in range(2):
                        nc.tensor.matmul(psum_s[:], ones_colr[:], expT[(h, H)][:],
                                         start=(H == 0), stop=(H == 1))
                    rs = rw.tile([1, QS], F32R, name="rs", tag="rs")
                    with nc.allow_low_precision(reason="f32r bits == f32; PE rounds anyway"):
                        nc.vector.reciprocal(rs[:], psum_s[:])
                    # broadcast 1/s to 32 partitions via PE (ones32^T @ rs)
                    rsb = psE1.tile([32, QS], F32, name="rsb", tag="rsb")
                    nc.tensor.matmul(rsb[:], ones32r[:], rs[:])

                    pav = psE1.tile([32, QS], F32, name="pav", tag="pav")
                    for H in range(2):
                        nc.tensor.matmul(pav[:], vT[H][:, 32 * h:32 * (h + 1)], expT[(h, H)][:],
                                         start=(H == 0), stop=(H == 1))
                    pav_sb = wk.tile([32, QS], F32, name="pav_sb", tag="pav_sb")
                    nc.scalar.copy(pav_sb[:], pav[:])
                    nc.vector.tensor_tensor(avn[32 * h:32 * (h + 1), :], pav_sb[:], rsb[:], ALU.mult)

                for m in range(2):
                    py = psE.tile([128, QS], F32, name="py", tag=f"psim0{m}")
                    nc.tensor.matmul(py[:], W(PK_WOT + m * 128, 128), avn[:])
                    y_sb = wk.tile([128, QS], F32, name="y_sb", tag="y_sb")
                    if m == 0:
                        nc.vector.tensor_copy(y_sb[:], py[:])
                        nc.sync.dma_start(y_out.ap()[m * 128:(m + 1) * 128, :], y_sb[:])
                    else:
                        nc.scalar.copy(y_sb[:], py[:])
                        nc.scalar.dma_start(y_out.ap()[m * 128:(m + 1) * 128, :], y_sb[:])

    nc.compile()
    return nc


def _shard_inputs(inputs):
    """Build the 8 per-core input maps from the full inputs."""
    x = np.ascontiguousarray(inputs["x"][0])               # [256, 1024]
    wq, wk, wv = inputs["wq"], inputs["wk"], inputs["wv"]  # [4, 64, 64]
    wo = inputs["wo"]                                      # [256, 256]
    w_off_dw = inputs["w_off_dw"][:, 0, :]                 # [64, 6]
    b_off_dw = inputs["b_off_dw"]                          # [64]
    w_off_proj = inputs["w_off_proj"]                      # [64]
    w1 = inputs["cpb_w1"][:, 0].astype(np.float64)         # [64]
    b1 = inputs["cpb_b1"].astype(np.float64)
    w2 = inputs["cpb_w2"].astype(np.float64)
    b2 = inputs["cpb_b2"].astype(np.float64)
    w3 = inputs["cpb_w3"].astype(np.float64)               # [2, 64]
    b3 = inputs["cpb_b3"].astype(np.float64)

    f = np.float32

    # bias lookup tables + Hankel strips, one per query-half (weights-only)
    slabs = {}
    for qh in range(2):
        kk = np.arange(LTAB, dtype=np.float64)
        d = HQ * (kk - CQ + QS * qh)
        pos = np.sign(d) * np.log1p(np.abs(d))
        h1 = np.maximum(pos[:, None] * w1[None, :] + b1, 0.0)
        h2 = np.maximum(h1 @ w2.T + b2, 0.0)
        T = (h2 @ w3.T + b3).astype(f)                     # [LTAB, 2]
        sl = np.zeros((128, 2 * 1536), f)
        for o in range(2):
            sw = np.lib.stride_tricks.sliding_window_view(T[:, o], 1536)  # [129, 1536]
            sl[:, o * 1536:(o + 1) * 1536] = sw[0:128]
        slabs[qh] = sl

    base_packed = np.zeros((128, PCK), f)
    base_packed[:, PK_WDW:PK_WDW + OFF_K] = np.concatenate([w_off_dw, w_off_dw], 0)
    base_packed[:, PK_BDW] = np.concatenate([b_off_dw, b_off_dw], 0)
    base_packed[0:DPG, PK_WPJ] = 0.5 * w_off_proj
    base_packed[DPG:128, PK_WPJ + 1] = 0.5 * w_off_proj
    base_packed[0:DPG, PK_EYE:PK_EYE + DPG] = np.eye(DPG, dtype=f)

    in_maps = []
    for c in range(NCORES):
        g, qh = c // 2, c % 2
        xg = np.ascontiguousarray(x[DPG * g:DPG * (g + 1)], dtype=f)
        xgT = np.zeros((128, NT * DPG), f)
        for t in range(NT):
            xgT[:, DPG * t:DPG * (t + 1)] = xg[:, 128 * t:128 * (t + 1)].T
        pk = base_packed.copy()
        pk[0:DPG, PK_WQTL:PK_WQTL + DPG] = wq[g].T
        pk[0:DPG, PK_WQTH + DPG:PK_WQTH + 128] = wq[g].T
        pk[0:DPG, PK_WKT:PK_WKT + DPG] = wk[g].T
        pk[0:DPG, PK_WVT:PK_WVT + DPG] = wv[g].T
        pk[0:DPG, PK_WOT:PK_WOT + DIM] = wo[:, DPG * g:DPG * (g + 1)].T
        pk[0:DPG, PK_WQTS:PK_WQTS + DPG] = wq[g].T * f(DH) ** f(-0.5)
        m = {
            "xg": xg,
            "xq": np.ascontiguousarray(xg[:, QS * qh:QS * (qh + 1)]),
            "xgT": xgT,
            "packed": pk,
            "hank": slabs[qh],
        }
        in_maps.append(m)
    return in_maps


def kernel(**inputs):
    if "nc" not in _CACHED:
        _CACHED["nc"] = build_nc()
    nc = _CACHED["nc"]
    in_maps = _shard_inputs(inputs)
    res = bass_utils.run_bass_kernel_spmd(nc, in_maps, core_ids=list(range(NCORES)))
    ys = [res.results[c]["y"] for c in range(NCORES)]
    bo = inputs["bo"]
    out = np.zeros((1, DIM, N), np.float32)
    for qh in range(2):
        acc = np.zeros((DIM, QS), np.float64)
        for g in range(G):
            acc += ys[2 * g + qh]
        out[0, :, QS * qh:QS * (qh + 1)] = (acc + bo.astype(np.float64)[:, None]).astype(np.float32)
    return out
